# revision 7
# baseline (speedup 1.0000x reference)
"""Decision Transformer forward pass on 8 Trainium2 NeuronCores.

Sharding: data-parallel over batch (32 -> 4 per core), model replicated.
Layout strategy per core:
  - residual x: fp32, token-major, lives in HBM ([BL, S, E])
  - LayerNorm token-major (bn_stats), output transposed via PE to
    feature-major hT (bf16) for matmuls
  - attention computed in transposed layout attT[t, s] (key-major) so the
    softmax denominator comes from a ones-matmul and attT feeds the o-matmul
    directly with v token-major as the stationary operand
  - softmax without max-subtraction (logits are tiny: ~N(0, 0.1)), scale
    1/sqrt(E) folded into the ACT exp, causal mask via affine_select on
    diagonal blocks only
  - FFN chunked over 512-token slabs, relu+bias fused in ACT eviction
  - 8 layers run under a hardware For_i loop with runtime-offset weight DMAs
"""
import numpy as np
import ml_dtypes

import concourse.bass as bass
import concourse.mybir as mybir
import concourse.tile as tile
from concourse import bacc
from concourse.bass import ds
from concourse.bass_utils import run_bass_kernel_spmd
from concourse.masks import make_identity

# model dims (hardcoded per problem spec)
B, T, E, H, OBS, ACTN, NB, MAXLEN = 32, 256, 512, 4, 128, 18, 8, 1024
HD = E // H            # 128
S = 3 * T              # 768
FF = 4 * E             # 2048
N_CORES = 8
BL = B // N_CORES      # 4 batch elems per core
ET = E // 128          # 4 e-tiles
FT = FF // 128         # 16 f-tiles
NTOK = BL * S          # 3072 tokens per core
NSEQT = S // 128       # 6 token tiles per sequence
SCALE = 1.0 / float(E) ** 0.5
FCH = 384              # ffn token chunk
NFCH = NTOK // FCH     # 6

F32 = mybir.dt.float32
F32R = mybir.dt.float32r
BF16 = mybir.dt.bfloat16
AF = mybir.ActivationFunctionType
ALU = mybir.AluOpType

DT = BF16              # matmul dtype for the transformer blocks


def build_nc(ln_trivial, bias_trivial, reps=1, unroll=False):
    nc = bacc.Bacc(target_bir_lowering=False, num_devices=N_CORES)

    # ---- dram parameters ----
    x0 = nc.declare_dram_parameter("x0", [BL, S, E], F32, isOutput=False)
    sT = nc.declare_dram_parameter("sT", [BL, OBS, T], F32R, isOutput=False)
    se_wT = nc.declare_dram_parameter("se_wT", [OBS, E], F32R, isOutput=False)
    wqk = nc.declare_dram_parameter("wqk", [NB, 2, H, ET, 128, HD], DT, isOutput=False)
    wv = nc.declare_dram_parameter("wv", [NB, ET, 128, E], DT, isOutput=False)
    wproj = nc.declare_dram_parameter("wproj", [NB, H, 128, E], DT, isOutput=False)
    wff1 = nc.declare_dram_parameter("wff1", [NB, ET, FT, 128, 128], DT, isOutput=False)
    wff2 = nc.declare_dram_parameter("wff2", [NB, FT, 128, E], DT, isOutput=False)
    # layer norm gains/biases, layer biases (always declared; ops conditional)
    lng = nc.declare_dram_parameter("lng", [NB, 2, E], F32, isOutput=False)  # ln1,ln2 g
    lnb = nc.declare_dram_parameter("lnb", [NB, 2, E], F32, isOutput=False)
    bproj = nc.declare_dram_parameter("bproj", [NB, E], F32, isOutput=False)
    bff1 = nc.declare_dram_parameter("bff1", [128, NB, FT], F32, isOutput=False)
    bff2 = nc.declare_dram_parameter("bff2", [NB, E], F32, isOutput=False)
    lnf_gb = nc.declare_dram_parameter("lnf_gb", [2, E], F32, isOutput=False)
    linSw = nc.declare_dram_parameter("linSw", [ET, 128, OBS], DT, isOutput=False)
    linAw = nc.declare_dram_parameter("linAw", [ET, 128, ACTN], DT, isOutput=False)
    linRw = nc.declare_dram_parameter("linRw", [1, E], F32, isOutput=False)
    hbias = nc.declare_dram_parameter("hbias", [1, OBS + ACTN + 1], F32, isOutput=False)

    oR = nc.declare_dram_parameter("oR", [BL, T, 1], F32, isOutput=True)
    oS = nc.declare_dram_parameter("oS", [BL, T, OBS], F32, isOutput=True)
    oA = nc.declare_dram_parameter("oA", [BL, T, ACTN], F32, isOutput=True)

    x_dram = nc.dram_tensor("x_res", [BL, S, E], F32)
    xm_dram = nc.dram_tensor("x_mid", [BL, S, E], F32)

    with tile.TileContext(nc) as tc:
        import contextlib
        with contextlib.ExitStack() as ctx:
            const = ctx.enter_context(tc.tile_pool(name="const", bufs=1))
            wpool = ctx.enter_context(tc.tile_pool(name="wpool", bufs=1))
            bpool = ctx.enter_context(tc.tile_pool(name="bpool", bufs=2))
            xio = ctx.enter_context(tc.tile_pool(name="xio", bufs=6))
            stat = ctx.enter_context(tc.tile_pool(name="stat", bufs=4))
            actp = ctx.enter_context(tc.tile_pool(name="actp", bufs=2))
            ps = ctx.enter_context(tc.tile_pool(name="ps", bufs=3, space="PSUM"))
            psmm = ctx.enter_context(tc.tile_pool(name="psmm", bufs=2, space="PSUM"))

            # ---- constants ----
            ident_f = const.tile([128, 128], F32)
            make_identity(nc, ident_f)
            ident = const.tile([128, 128], DT)
            nc.vector.tensor_copy(ident, ident_f)
            ones_k = const.tile([128, 1], DT)
            nc.vector.memset(ones_k, 1.0)
            eps_t = const.tile([128, 1], F32)
            nc.vector.memset(eps_t, 1e-5)

            sew_sb = const.tile([OBS, E], F32R)
            nc.sync.dma_start(sew_sb, se_wT[:])

            # =========================================================
            # helpers
            # =========================================================
            def layernorm_tile(x_t, g_bc, b_bc, out_dt, out_ap, trivial):
                """x_t: [128, E] fp32 sbuf -> out_ap [128, E] out_dt normalized."""
                st6 = stat.tile([128, 6], F32, tag="bn6")
                nc.vector.bn_stats(st6, x_t)
                mv = stat.tile([128, 2], F32, tag="mv")
                nc.vector.bn_aggr(mv, st6)
                rstd = stat.tile([128, 1], F32, tag="rstd")
                nc.scalar.activation(rstd, mv[:, 1:2], AF.Sqrt, bias=eps_t, scale=1.0)
                nc.vector.reciprocal(rstd, rstd)
                negmr = stat.tile([128, 1], F32, tag="negmr")
                nc.vector.tensor_scalar(negmr, mv[:, 0:1], rstd, -1.0,
                                        op0=ALU.mult, op1=ALU.mult)
                if trivial:
                    nc.scalar.activation(out_ap, x_t, AF.Identity,
                                         bias=negmr, scale=rstd)
                else:
                    xh = stat.tile([128, E], F32, tag="xhat", bufs=2)
                    nc.scalar.activation(xh, x_t, AF.Identity, bias=negmr, scale=rstd)
                    xh2 = stat.tile([128, E], F32, tag="xhat2", bufs=2)
                    nc.vector.tensor_tensor(xh2, xh, g_bc, op=ALU.mult)
                    nc.vector.tensor_tensor(out_ap, xh2, b_bc, op=ALU.add)

            def transpose_to(dst_ap, src_ap, dt):
                """src [128, 128] dt -> dst [128, 128] dt (transposed)."""
                pt = ps.tile([128, 128], dt, tag="tp", bufs=2)
                nc.tensor.transpose(pt, src_ap, ident)
                nc.vector.tensor_copy(dst_ap, pt)

            def load_bcast(row_ap, width, tag, engine_pool=bpool):
                """DMA a [1, width] dram row into sbuf and broadcast to 128 parts."""
                row = stat.tile([1, width], F32, tag=tag + "_row")
                nc.sync.dma_start(row, row_ap)
                bc = engine_pool.tile([128, width], F32, tag=tag)
                nc.gpsimd.partition_broadcast(bc, row)
                return bc

            # =========================================================
            # embedding: x0 -> x_dram ; x[3t+1] += sT.T @ se_wT
            # =========================================================
            nc.sync.dma_start(x_dram[:], x0[:])
            for b in range(BL):
                s1rows = x_dram[b].rearrange("(t three) e -> three t e", three=3)[1]
                sT_sb = xio.tile([128, T], F32R, tag="sTsb")
                nc.sync.dma_start(sT_sb, sT[b])
                for tt in range(T // 128):
                    pse = psmm.tile([128, E], F32, tag="mm")
                    nc.tensor.matmul(pse, sT_sb[:, tt * 128:(tt + 1) * 128],
                                     sew_sb, start=True, stop=True)
                    xrow = xio.tile([128, E], F32, tag="xio")
                    nc.sync.dma_start(xrow, s1rows[tt * 128:(tt + 1) * 128, :])
                    xrow2 = xio.tile([128, E], F32, tag="xio")
                    nc.vector.tensor_tensor(xrow2, xrow, pse, op=ALU.add)
                    nc.sync.dma_start(s1rows[tt * 128:(tt + 1) * 128, :], xrow2)

            # =========================================================
            # transformer layer
            # =========================================================
            def layer_body(i):
                # ---- per-layer weights ----
                wqk_sb = wpool.tile([128, 2, H, ET, HD], DT, tag="wqk")
                nc.sync.dma_start(wqk_sb, wqk[ds(i, 1)].rearrange(
                    "o a h e p d -> (o p) a h e d"))
                wv_sb = wpool.tile([128, ET, E], DT, tag="wv")
                nc.sync.dma_start(wv_sb, wv[ds(i, 1)].rearrange(
                    "o e p n -> (o p) e n"))
                wp_sb = wpool.tile([128, H, E], DT, tag="wproj")
                nc.sync.dma_start(wp_sb, wproj[ds(i, 1)].rearrange(
                    "o h p n -> (o p) h n"))
                wf1_sb = wpool.tile([128, ET, FT, 128], DT, tag="wff1")
                nc.sync.dma_start(wf1_sb, wff1[ds(i, 1)].rearrange(
                    "o e f p d -> (o p) e f d"))
                wf2_sb = wpool.tile([128, FT, E], DT, tag="wff2")
                nc.sync.dma_start(wf2_sb, wff2[ds(i, 1)].rearrange(
                    "o f p n -> (o p) f n"))

                if not ln_trivial:
                    g1_bc = load_bcast(lng[ds(i, 1)][0, 0:1, :], E, "g1")
                    b1_bc = load_bcast(lnb[ds(i, 1)][0, 0:1, :], E, "b1")
                    g2_bc = load_bcast(lng[ds(i, 1)][0, 1:2, :], E, "g2")
                    b2_bc = load_bcast(lnb[ds(i, 1)][0, 1:2, :], E, "b2")
                else:
                    g1_bc = b1_bc = g2_bc = b2_bc = None
                if not bias_trivial:
                    bp_bc = load_bcast(bproj[ds(i, 1)], E, "bp")
                    bf2_bc = load_bcast(bff2[ds(i, 1)], E, "bf2")
                    bf1_l = bpool.tile([128, FT], F32, tag="bf1")
                    nc.sync.dma_start(bf1_l, bff1[:, ds(i, 1), :].rearrange(
                        "p o f -> p (o f)"))
                else:
                    bp_bc = bf2_bc = bf1_l = None

                # ---- attention over each sequence ----
                for b in range(BL):
                    # LN1 + transpose -> hT [128, ET, S]
                    hT = actp.tile([128, ET, S], DT, tag="hT")
                    for j in range(NSEQT):
                        x_t = xio.tile([128, E], F32, tag="xio")
                        nc.sync.dma_start(x_t, x_dram[b, j * 128:(j + 1) * 128, :])
                        h_t = stat.tile([128, E], DT, tag="hh", bufs=3, name="h1")
                        layernorm_tile(x_t, g1_bc, b1_bc, DT, h_t, ln_trivial)
                        for e in range(ET):
                            transpose_to(hT[:, e, j * 128:(j + 1) * 128],
                                         h_t[:, e * 128:(e + 1) * 128], DT)

                    # QKV
                    qT = actp.tile([128, H, S], DT, tag="qT", bufs=1)
                    kT = actp.tile([128, H, S], DT, tag="kT", bufs=1)
                    for h in range(H):
                        for sc in range(2):
                            w0 = sc * 384
                            for qk in range(2):
                                pq = ps.tile([128, 384], F32, tag="ps")
                                for e in range(ET):
                                    nc.tensor.matmul(
                                        pq, wqk_sb[:, qk, h, e, :],
                                        hT[:, e, w0:w0 + 384],
                                        start=(e == 0), stop=(e == ET - 1))
                                dst = (qT if qk == 0 else kT)[:, h, w0:w0 + 384]
                                nc.scalar.copy(dst, pq)
                    v_sb = actp.tile([128, NSEQT, E], DT, tag="v", bufs=1)
                    for j in range(NSEQT):
                        pv = ps.tile([128, E], F32, tag="ps")
                        for e in range(ET):
                            nc.tensor.matmul(pv, hT[:, e, j * 128:(j + 1) * 128],
                                             wv_sb[:, e, :],
                                             start=(e == 0), stop=(e == ET - 1))
                        nc.scalar.copy(v_sb[:, j, :], pv)

                    # attention per head
                    oT = actp.tile([128, H, S], DT, tag="oT", bufs=1)
                    for h in range(H):
                        att = [actp.tile([128, S - 128 * t_], DT, tag=f"att{t_}", bufs=1,
                                         name=f"att{t_}")
                               for t_ in range(NSEQT)]
                        recip_row = stat.tile([1, S], F32, tag="recip_row")
                        for sc in range(2):
                            s0 = sc * 384
                            ntt = 3 * sc + 3
                            # scores + exp (+ causal mask on diagonal blocks)
                            for tt_ in range(ntt):
                                s_lo = max(s0, 128 * tt_)
                                wdt = s0 + 384 - s_lo
                                pa = ps.tile([128, 384], F32, tag="ps")
                                nc.tensor.matmul(
                                    pa[:, :wdt], kT[:, h, tt_ * 128:(tt_ + 1) * 128],
                                    qT[:, h, s_lo:s_lo + wdt],
                                    start=True, stop=True)
                                dst = att[tt_][:, s_lo - 128 * tt_:
                                               s_lo - 128 * tt_ + wdt]
                                nc.scalar.activation(dst, pa[:, :wdt], AF.Exp,
                                                     scale=SCALE)
                                if s_lo == 128 * tt_:  # diagonal block
                                    nc.gpsimd.affine_select(
                                        dst, dst, pattern=[[1, wdt]],
                                        compare_op=ALU.is_ge, fill=0.0,
                                        base=0, channel_multiplier=-1)
                            # denominator via ones-matmul
                            pd = ps.tile([1, 384], F32, tag="dn", bufs=1)
                            for tt_ in range(ntt):
                                s_lo = max(s0, 128 * tt_)
                                wdt = s0 + 384 - s_lo
                                nc.tensor.matmul(
                                    pd[:, s_lo - s0:s_lo - s0 + wdt], ones_k,
                                    att[tt_][:, s_lo - 128 * tt_:
                                             s_lo - 128 * tt_ + wdt],
                                    start=(tt_ == 0), stop=(tt_ == ntt - 1),
                                    skip_group_check=True)
                            nc.vector.reciprocal(recip_row[:, s0:s0 + 384], pd)
                        recip_bc = actp.tile([128, S], F32, tag="recip_bc")
                        nc.gpsimd.partition_broadcast(recip_bc, recip_row)
                        # o = v.T @ att, normalized on eviction
                        for sc in range(2):
                            s0 = sc * 384
                            ntt = 3 * sc + 3
                            po = ps.tile([128, 384], F32, tag="ps")
                            for tt_ in range(ntt):
                                s_lo = max(s0, 128 * tt_)
                                wdt = s0 + 384 - s_lo
                                nc.tensor.matmul(
                                    po[:, s_lo - s0:s_lo - s0 + wdt],
                                    v_sb[:, tt_, h * HD:(h + 1) * HD],
                                    att[tt_][:, s_lo - 128 * tt_:
                                             s_lo - 128 * tt_ + wdt],
                                    start=(tt_ == 0), stop=(tt_ == ntt - 1),
                                    skip_group_check=True)
                            nc.vector.tensor_tensor(oT[:, h, s0:s0 + 384], po,
                                                    recip_bc[:, s0:s0 + 384],
                                                    op=ALU.mult)

                    # proj + residual -> xm_dram
                    for j in range(NSEQT):
                        pp = psmm.tile([128, E], F32, tag="mm")
                        for h in range(H):
                            nc.tensor.matmul(pp, oT[:, h, j * 128:(j + 1) * 128],
                                             wp_sb[:, h, :],
                                             start=(h == 0), stop=(h == H - 1))
                        x_t = xio.tile([128, E], F32, tag="xio")
                        nc.sync.dma_start(x_t, x_dram[b, j * 128:(j + 1) * 128, :])
                        xm_t = xio.tile([128, E], F32, tag="xio")
                        if bias_trivial:
                            nc.vector.tensor_tensor(xm_t, pp, x_t, op=ALU.add)
                        else:
                            nc.vector.tensor_tensor(xm_t, pp, bp_bc, op=ALU.add)
                            nc.vector.tensor_tensor(xm_t, xm_t, x_t, op=ALU.add)
                        nc.sync.dma_start(xm_dram[b, j * 128:(j + 1) * 128, :], xm_t)

                # ---- FFN over 512-token chunks ----
                xm_flat = xm_dram[:].rearrange("b s e -> (b s) e")
                x_flat = x_dram[:].rearrange("b s e -> (b s) e")
                for ch in range(NFCH):
                    c0 = ch * FCH
                    h2T = actp.tile([128, ET, FCH], DT, tag="h2T")
                    for j in range(FCH // 128):
                        xm_t = xio.tile([128, E], F32, tag="xio")
                        nc.sync.dma_start(
                            xm_t, xm_flat[c0 + j * 128:c0 + (j + 1) * 128, :])
                        h2_t = stat.tile([128, E], DT, tag="hh", bufs=3, name="h2")
                        layernorm_tile(xm_t, g2_bc, b2_bc, DT, h2_t, ln_trivial)
                        for e in range(ET):
                            transpose_to(h2T[:, e, j * 128:(j + 1) * 128],
                                         h2_t[:, e * 128:(e + 1) * 128], DT)
                    uT = actp.tile([128, FT, FCH], DT, tag="uT")
                    for f in range(FT):
                        pu = ps.tile([128, FCH], F32, tag="ps")
                        for e in range(ET):
                            nc.tensor.matmul(pu, wf1_sb[:, e, f, :], h2T[:, e, :],
                                             start=(e == 0), stop=(e == ET - 1))
                        if bias_trivial:
                            nc.scalar.activation(uT[:, f, :], pu, AF.Relu)
                        else:
                            nc.scalar.activation(uT[:, f, :], pu, AF.Relu,
                                                 bias=bf1_l[:, f:f + 1])
                    for j in range(FCH // 128):
                        py = psmm.tile([128, E], F32, tag="mm")
                        for f in range(FT):
                            nc.tensor.matmul(py, uT[:, f, j * 128:(j + 1) * 128],
                                             wf2_sb[:, f, :],
                                             start=(f == 0), stop=(f == FT - 1))
                        xm_t = xio.tile([128, E], F32, tag="xio")
                        nc.sync.dma_start(
                            xm_t, xm_flat[c0 + j * 128:c0 + (j + 1) * 128, :])
                        xo_t = xio.tile([128, E], F32, tag="xio")
                        if bias_trivial:
                            nc.vector.tensor_tensor(xo_t, py, xm_t, op=ALU.add)
                        else:
                            nc.vector.tensor_tensor(xo_t, py, bf2_bc, op=ALU.add)
                            nc.vector.tensor_tensor(xo_t, xo_t, xm_t, op=ALU.add)
                        nc.sync.dma_start(
                            x_flat[c0 + j * 128:c0 + (j + 1) * 128, :], xo_t)

            if unroll:
                for i in range(NB):
                    layer_body(i)
            else:
                with tc.For_i(0, NB, 1) as i:
                    layer_body(i)

            # =========================================================
            # final layernorm + heads
            # =========================================================
            if not ln_trivial:
                gf_bc = load_bcast(lnf_gb[0:1, :], E, "gf", const)
                bf_bc = load_bcast(lnf_gb[1:2, :], E, "bf", const)
            else:
                gf_bc = bf_bc = None
            linS_sb = const.tile([128, ET, OBS], DT)
            nc.sync.dma_start(linS_sb, linSw[:].rearrange("e p n -> p e n"))
            linA_sb = const.tile([128, ET, ACTN], DT)
            nc.sync.dma_start(linA_sb, linAw[:].rearrange("e p n -> p e n"))
            linRw_bc = load_bcast(linRw[0:1, :], E, "linrw", const)
            hb_bc = load_bcast(hbias[0:1, :], OBS + ACTN + 1, "hbias", const)

            for b in range(BL):
                xv = x_dram[b].rearrange("(t three) e -> three t e", three=3)
                for stream, head in ((2, "RS"), (1, "A")):
                    for j in range(T // 128):
                        x_t = xio.tile([128, E], F32, tag="xio")
                        nc.sync.dma_start(x_t, xv[stream, j * 128:(j + 1) * 128, :])
                        xf = stat.tile([128, E], F32, tag="xf", bufs=2)
                        layernorm_tile(x_t, gf_bc, bf_bc, F32, xf, ln_trivial)
                        if not ln_trivial:
                            pass  # layernorm_tile already applied g/b
                        if head == "RS":
                            # R head on DVE: sum(xf * w) along free dim
                            tmp = stat.tile([128, E], F32, tag="rtmp", bufs=2)
                            yR = stat.tile([128, 1], F32, tag="yR")
                            nc.vector.scalar_tensor_tensor(
                                tmp, xf, 1.0, linRw_bc,
                                op0=ALU.mult, op1=ALU.mult, accum_out=yR)
                            yRb = stat.tile([128, 1], F32, tag="yRb")
                            nc.vector.tensor_tensor(
                                yRb, yR, hb_bc[:, OBS + ACTN:OBS + ACTN + 1],
                                op=ALU.add)
                            nc.sync.dma_start(oR[b, j * 128:(j + 1) * 128, :], yRb)
                        # transpose xf (cast to DT) for the matmul heads
                        xf_c = stat.tile([128, E], DT, tag="xfc", bufs=2)
                        nc.vector.tensor_copy(xf_c, xf)
                        xfT = stat.tile([128, ET, 128], DT, tag="xfT", bufs=2)
                        for e in range(ET):
                            transpose_to(xfT[:, e, :],
                                         xf_c[:, e * 128:(e + 1) * 128], DT)
                        if head == "RS":
                            pS = psmm.tile([128, OBS], F32, tag="mm")
                            for e in range(ET):
                                nc.tensor.matmul(pS, xfT[:, e, :], linS_sb[:, e, :],
                                                 start=(e == 0), stop=(e == ET - 1))
                            yS = stat.tile([128, OBS], F32, tag="yS")
                            nc.vector.tensor_tensor(yS, pS, hb_bc[:, :OBS],
                                                    op=ALU.add)
                            nc.sync.dma_start(oS[b, j * 128:(j + 1) * 128, :], yS)
                        else:
                            pA = psmm.tile([128, ACTN], F32, tag="mm")
                            for e in range(ET):
                                nc.tensor.matmul(pA, xfT[:, e, :], linA_sb[:, e, :],
                                                 start=(e == 0), stop=(e == ET - 1))
                            yA = stat.tile([128, ACTN], F32, tag="yA")
                            nc.vector.tensor_tensor(
                                yA, pA, hb_bc[:, OBS:OBS + ACTN], op=ALU.add)
                            nc.sync.dma_start(oA[b, j * 128:(j + 1) * 128, :], yA)

    nc.compile()
    return nc


_BUILD_CACHE = {}


def _get_nc(ln_trivial, bias_trivial):
    key = (ln_trivial, bias_trivial)
    if key not in _BUILD_CACHE:
        _BUILD_CACHE[key] = build_nc(ln_trivial, bias_trivial)
    return _BUILD_CACHE[key]


def _prep_weights(inputs):
    f32 = lambda x: np.ascontiguousarray(np.asarray(x, dtype=np.float32))
    bf = lambda x: np.ascontiguousarray(
        np.asarray(x, dtype=np.float32).astype(ml_dtypes.bfloat16))
    wq, wk, wv_, pw = (f32(inputs[k]) for k in ("wq", "wk", "wv", "proj_w"))
    ff1, ff2 = f32(inputs["ff1_w"]), f32(inputs["ff2_w"])

    # wqk[i, a, h, e, p, d] = w[i, h, d, e*128+p]
    wqk_np = np.stack([wq, wk], axis=1)          # [NB, 2, H, HD, E]
    wqk_np = wqk_np.transpose(0, 1, 2, 4, 3)      # [NB, 2, H, E, HD]
    wqk_np = wqk_np.reshape(NB, 2, H, ET, 128, HD)
    # wv[i, e, p, n] with n = h*HD+d : wv_[i,h,d,e128+p]
    wv_np = wv_.reshape(NB, E, E).transpose(0, 2, 1).reshape(NB, ET, 128, E)
    # wproj[i, h, p, n] = proj_w[i, n, h*128+p]  (rhs [e, o])
    wp_np = pw.transpose(0, 2, 1).reshape(NB, ET, 128, E)
    # wff1[i, e, f, p, d] = ff1[i, f*128+d, e*128+p]
    wff1_np = ff1.transpose(0, 2, 1).reshape(NB, ET, 128, FF) \
        .reshape(NB, ET, 128, FT, 128).transpose(0, 1, 3, 2, 4)
    # wff2[i, f, p, n] = ff2[i, n, f*128+p]
    wff2_np = ff2.transpose(0, 2, 1).reshape(NB, FT, 128, E)
    # bff1[p, i, f] = ff1_b[i, f*128+p]
    bff1_np = f32(inputs["ff1_b"]).reshape(NB, FT, 128).transpose(2, 0, 1)

    linS_np = f32(inputs["linS_w"]).T.reshape(ET, 128, OBS)   # [e,p,obs]
    linA_np = f32(inputs["linA_w"]).T.reshape(ET, 128, ACTN)

    return {
        "se_wT": f32(inputs["se_w"]).T.copy(),                # [OBS, E]
        "wqk": bf(wqk_np), "wv": bf(wv_np), "wproj": bf(wp_np),
        "wff1": bf(wff1_np), "wff2": bf(wff2_np),
        "lng": np.stack([f32(inputs["ln1_g"]), f32(inputs["ln2_g"])], 1),
        "lnb": np.stack([f32(inputs["ln1_b"]), f32(inputs["ln2_b"])], 1),
        "bproj": f32(inputs["proj_b"]), "bff1": bff1_np,
        "bff2": f32(inputs["ff2_b"]),
        "lnf_gb": np.stack([f32(inputs["lnf_g"]), f32(inputs["lnf_b"])], 0),
        "linSw": bf(linS_np), "linAw": bf(linA_np),
        "linRw": f32(inputs["linR_w"]).reshape(1, E),
        "hbias": np.concatenate([f32(inputs["linS_b"]), f32(inputs["linA_b"]),
                                 f32(inputs["linR_b"])]).reshape(1, -1),
    }


def _detect_flags(inputs):
    ln_trivial = all(
        np.all(np.asarray(inputs[k], np.float32) == 1.0) for k in
        ("ln1_g", "ln2_g", "lnf_g")) and all(
        np.all(np.asarray(inputs[k], np.float32) == 0.0) for k in
        ("ln1_b", "ln2_b", "lnf_b"))
    bias_trivial = all(
        np.all(np.asarray(inputs[k], np.float32) == 0.0) for k in
        ("proj_b", "ff1_b", "ff2_b"))
    return ln_trivial, bias_trivial


def _make_in_maps(inputs):
    R = np.asarray(inputs["R"], dtype=np.float32)
    s = np.asarray(inputs["s"], dtype=np.float32)
    a = np.asarray(inputs["a"]).astype(np.int64)
    t = np.asarray(inputs["t"]).astype(np.int64)
    pos_emb = np.asarray(inputs["pos_emb"], dtype=np.float32)
    act_emb = np.asarray(inputs["act_emb"], dtype=np.float32)

    w = _prep_weights(inputs)

    # host-side embedding prep (pure gathers / elementwise; the se matmul
    # runs on device)
    pos = pos_emb[t]                                   # [B, T, E]
    re_w = np.asarray(inputs["re_w"], dtype=np.float32)
    re_b = np.asarray(inputs["re_b"], dtype=np.float32)
    se_b = np.asarray(inputs["se_b"], dtype=np.float32)
    re = R * re_w[:, 0][None, None, :] + re_b + pos    # [B, T, E]
    ae = act_emb[a] + pos
    se_part = se_b[None, None, :] + pos                # se matmul added on device
    x0 = np.stack([re, se_part, ae], axis=2).reshape(B, S, E).astype(np.float32)

    head_bias = np.concatenate([
        np.asarray(inputs["linS_b"], np.float32).ravel(),
        np.asarray(inputs["linA_b"], np.float32).ravel(),
        np.asarray(inputs["linR_b"], np.float32).ravel()]).reshape(1, -1)
    w["hbias"] = head_bias

    in_maps = []
    for c in range(N_CORES):
        bsl = slice(c * BL, (c + 1) * BL)
        m = dict(w)
        m["x0"] = np.ascontiguousarray(x0[bsl])
        m["sT"] = np.ascontiguousarray(s[bsl].transpose(0, 2, 1))
        in_maps.append(m)
    return in_maps


def kernel(**inputs):
    in_maps = _make_in_maps(inputs)
    nc = _get_nc(*_detect_flags(inputs))
    res = run_bass_kernel_spmd(nc, in_maps, core_ids=list(range(N_CORES)))
    outR = np.concatenate([res.results[c]["oR"] for c in range(N_CORES)], 0)
    outS = np.concatenate([res.results[c]["oS"] for c in range(N_CORES)], 0)
    outA = np.concatenate([res.results[c]["oA"] for c in range(N_CORES)], 0)
    return outR, outS, outA


# revision 11
# speedup vs baseline: 25953985.4148x; 25953985.4148x over previous
"""Decision Transformer forward pass on 8 Trainium2 NeuronCores.

Sharding: data-parallel over batch (32 -> 4 per core), model replicated.
Layout strategy per core:
  - residual x: fp32, token-major, lives in HBM ([BL, S, E])
  - LayerNorm token-major (bn_stats), output transposed via PE to
    feature-major hT (bf16) for matmuls
  - attention computed in transposed layout attT[t, s] (key-major) so the
    softmax denominator comes from a ones-matmul and attT feeds the o-matmul
    directly with v token-major as the stationary operand
  - softmax without max-subtraction (logits are tiny: ~N(0, 0.1)), scale
    1/sqrt(E) folded into the ACT exp, causal mask via affine_select on
    diagonal blocks only
  - FFN chunked over 512-token slabs, relu+bias fused in ACT eviction
  - 8 layers run under a hardware For_i loop with runtime-offset weight DMAs
"""
import numpy as np
import ml_dtypes

import concourse.bass as bass
import concourse.mybir as mybir
import concourse.tile as tile
from concourse import bacc
from concourse.bass import ds
from concourse.bass_utils import run_bass_kernel_spmd
from concourse.masks import make_identity

# model dims (hardcoded per problem spec)
B, T, E, H, OBS, ACTN, NB, MAXLEN = 32, 256, 512, 4, 128, 18, 8, 1024
HD = E // H            # 128
S = 3 * T              # 768
FF = 4 * E             # 2048
N_CORES = 8
BL = B // N_CORES      # 4 batch elems per core
ET = E // 128          # 4 e-tiles
FT = FF // 128         # 16 f-tiles
NTOK = BL * S          # 3072 tokens per core
NSEQT = S // 128       # 6 token tiles per sequence
SCALE = 1.0 / float(E) ** 0.5
FCH = 384              # ffn token chunk
NFCH = NTOK // FCH     # 6

F32 = mybir.dt.float32
F32R = mybir.dt.float32r
BF16 = mybir.dt.bfloat16
AF = mybir.ActivationFunctionType
ALU = mybir.AluOpType

DT = BF16              # matmul dtype for the transformer blocks


def build_nc(ln_trivial, bias_trivial, reps=1, unroll=False):
    nc = bacc.Bacc(target_bir_lowering=False, num_devices=N_CORES)

    # ---- dram parameters ----
    x0 = nc.declare_dram_parameter("x0", [BL, S, E], F32, isOutput=False)
    sT = nc.declare_dram_parameter("sT", [BL, OBS, T], F32R, isOutput=False)
    se_wT = nc.declare_dram_parameter("se_wT", [OBS, E], F32R, isOutput=False)
    wqk = nc.declare_dram_parameter("wqk", [NB, 128, 2 * H * ET * HD], DT, isOutput=False)
    wv = nc.declare_dram_parameter("wv", [NB, 128, ET * E], DT, isOutput=False)
    wproj = nc.declare_dram_parameter("wproj", [NB, 128, H * E], DT, isOutput=False)
    wff1 = nc.declare_dram_parameter("wff1", [NB, 128, ET * FT * 128], DT, isOutput=False)
    wff2 = nc.declare_dram_parameter("wff2", [NB, 128, FT * E], DT, isOutput=False)
    # layer norm gains/biases, layer biases (always declared; ops conditional)
    lng = nc.declare_dram_parameter("lng", [NB, 2, E], F32, isOutput=False)  # ln1,ln2 g
    lnb = nc.declare_dram_parameter("lnb", [NB, 2, E], F32, isOutput=False)
    bproj = nc.declare_dram_parameter("bproj", [NB, E], F32, isOutput=False)
    bff1 = nc.declare_dram_parameter("bff1", [128, NB, FT], F32, isOutput=False)
    bff2 = nc.declare_dram_parameter("bff2", [NB, E], F32, isOutput=False)
    lnf_gb = nc.declare_dram_parameter("lnf_gb", [2, E], F32, isOutput=False)
    linSw = nc.declare_dram_parameter("linSw", [128, ET * OBS], DT, isOutput=False)
    linAw = nc.declare_dram_parameter("linAw", [128, ET * ACTN], DT, isOutput=False)
    linRw = nc.declare_dram_parameter("linRw", [1, E], F32, isOutput=False)
    hbias = nc.declare_dram_parameter("hbias", [1, OBS + ACTN + 1], F32, isOutput=False)

    oR = nc.declare_dram_parameter("oR", [BL, T, 1], F32, isOutput=True)
    oS = nc.declare_dram_parameter("oS", [BL, T, OBS], F32, isOutput=True)
    oA = nc.declare_dram_parameter("oA", [BL, T, ACTN], F32, isOutput=True)

    x_dram = nc.dram_tensor("x_scratch", [BL, S, E], F32)

    with tile.TileContext(nc) as tc:
        import contextlib
        with contextlib.ExitStack() as ctx:
            const = ctx.enter_context(tc.tile_pool(name="const", bufs=1))
            wpool = ctx.enter_context(tc.tile_pool(name="wpool", bufs=1))
            bpool = ctx.enter_context(tc.tile_pool(name="bpool", bufs=2))
            xio = ctx.enter_context(tc.tile_pool(name="xio", bufs=3))
            stat = ctx.enter_context(tc.tile_pool(name="stat", bufs=4))
            actp = ctx.enter_context(tc.tile_pool(name="actp", bufs=2))
            ps = ctx.enter_context(tc.tile_pool(name="ps", bufs=3, space="PSUM"))
            psmm = ctx.enter_context(tc.tile_pool(name="psmm", bufs=2, space="PSUM"))

            # ---- constants ----
            ident_f = const.tile([128, 128], F32)
            make_identity(nc, ident_f)
            ident = const.tile([128, 128], DT)
            nc.vector.tensor_copy(ident, ident_f)
            ones_k = const.tile([128, 1], DT)
            nc.vector.memset(ones_k, 1.0)
            eps_t = const.tile([128, 1], F32)
            nc.vector.memset(eps_t, 1e-5)

            sew_sb = const.tile([OBS, E], F32R)
            nc.sync.dma_start(sew_sb, se_wT[:])

            x_res = [const.tile([128, E], F32, name=f"xres{k}", tag=f"xres{k}")
                     for k in range(BL * NSEQT)]

            # =========================================================
            # helpers
            # =========================================================
            def layernorm_tile(x_t, g_bc, b_bc, out_dt, out_ap, trivial):
                """x_t: [128, E] fp32 sbuf -> out_ap [128, E] out_dt normalized."""
                st6 = stat.tile([128, 6], F32, tag="bn6")
                nc.vector.bn_stats(st6, x_t)
                mv = stat.tile([128, 2], F32, tag="mv")
                nc.vector.bn_aggr(mv, st6)
                rstd = stat.tile([128, 1], F32, tag="rstd")
                nc.scalar.activation(rstd, mv[:, 1:2], AF.Sqrt, bias=eps_t, scale=1.0)
                nc.vector.reciprocal(rstd, rstd)
                negmr = stat.tile([128, 1], F32, tag="negmr")
                nc.vector.tensor_scalar(negmr, mv[:, 0:1], rstd, -1.0,
                                        op0=ALU.mult, op1=ALU.mult)
                if trivial:
                    nc.scalar.activation(out_ap, x_t, AF.Identity,
                                         bias=negmr, scale=rstd)
                else:
                    xh = stat.tile([128, E], F32, tag="xhat", bufs=2)
                    nc.scalar.activation(xh, x_t, AF.Identity, bias=negmr, scale=rstd)
                    xh2 = stat.tile([128, E], F32, tag="xhat2", bufs=2)
                    nc.vector.tensor_tensor(xh2, xh, g_bc, op=ALU.mult)
                    nc.vector.tensor_tensor(out_ap, xh2, b_bc, op=ALU.add)

            def transpose_to(dst_ap, src_ap, dt):
                """src [128, 128] dt -> dst [128, 128] dt (transposed)."""
                pt = ps.tile([128, 128], dt, tag="tp", bufs=2)
                nc.tensor.transpose(pt, src_ap, ident)
                nc.vector.tensor_copy(dst_ap, pt)

            def load_bcast(row_ap, width, tag, engine_pool=bpool):
                """DMA a [1, width] dram row into sbuf and broadcast to 128 parts."""
                row = stat.tile([1, width], F32, tag=tag + "_row")
                nc.sync.dma_start(row, row_ap)
                bc = engine_pool.tile([128, width], F32, tag=tag)
                nc.gpsimd.partition_broadcast(bc, row)
                return bc

            # =========================================================
            # embedding: x0 -> x_dram ; x[3t+1] += sT.T @ se_wT
            # =========================================================
            nc.sync.dma_start(x_dram[:], x0[:])
            for b in range(BL):
                s1rows = x_dram[b].rearrange("(t three) e -> three t e", three=3)[1]
                sT_sb = xio.tile([128, T], F32R, tag="sTsb")
                nc.sync.dma_start(sT_sb, sT[b])
                for tt in range(T // 128):
                    pse = psmm.tile([128, E], F32, tag="mm")
                    nc.tensor.matmul(pse, sT_sb[:, tt * 128:(tt + 1) * 128],
                                     sew_sb, start=True, stop=True)
                    xrow = xio.tile([128, E], F32, tag="xio")
                    nc.sync.dma_start(xrow, s1rows[tt * 128:(tt + 1) * 128, :])
                    xrow2 = xio.tile([128, E], F32, tag="xio")
                    nc.vector.tensor_tensor(xrow2, xrow, pse, op=ALU.add)
                    nc.sync.dma_start(s1rows[tt * 128:(tt + 1) * 128, :], xrow2)
            for k in range(BL * NSEQT):
                b_, j_ = divmod(k, NSEQT)
                nc.sync.dma_start(x_res[k],
                                  x_dram[b_, j_ * 128:(j_ + 1) * 128, :])

            # =========================================================
            # transformer layer
            # =========================================================
            def layer_body(i):
                # ---- per-layer weights ----
                wqk_sb = wpool.tile([128, 2, H, ET, HD], DT, tag="wqk")
                nc.sync.dma_start(
                    wqk_sb.rearrange("p a h e d -> p (a h e d)"),
                    wqk[ds(i, 1)].rearrange("o p x -> (o p) x"))
                wv_sb = wpool.tile([128, ET, E], DT, tag="wv")
                nc.sync.dma_start(wv_sb.rearrange("p e n -> p (e n)"),
                                  wv[ds(i, 1)].rearrange("o p x -> (o p) x"))
                wp_sb = wpool.tile([128, H, E], DT, tag="wproj")
                nc.sync.dma_start(wp_sb.rearrange("p h n -> p (h n)"),
                                  wproj[ds(i, 1)].rearrange("o p x -> (o p) x"))
                wf1_sb = wpool.tile([128, ET, FT, 128], DT, tag="wff1")
                nc.sync.dma_start(wf1_sb.rearrange("p e f d -> p (e f d)"),
                                  wff1[ds(i, 1)].rearrange("o p x -> (o p) x"))
                wf2_sb = wpool.tile([128, FT, E], DT, tag="wff2")
                nc.sync.dma_start(wf2_sb.rearrange("p f n -> p (f n)"),
                                  wff2[ds(i, 1)].rearrange("o p x -> (o p) x"))

                if not ln_trivial:
                    g1_bc = load_bcast(lng[ds(i, 1)][0, 0:1, :], E, "g1")
                    b1_bc = load_bcast(lnb[ds(i, 1)][0, 0:1, :], E, "b1")
                    g2_bc = load_bcast(lng[ds(i, 1)][0, 1:2, :], E, "g2")
                    b2_bc = load_bcast(lnb[ds(i, 1)][0, 1:2, :], E, "b2")
                else:
                    g1_bc = b1_bc = g2_bc = b2_bc = None
                if not bias_trivial:
                    bp_bc = load_bcast(bproj[ds(i, 1)], E, "bp")
                    bf2_bc = load_bcast(bff2[ds(i, 1)], E, "bf2")
                    bf1_l = bpool.tile([128, FT], F32, tag="bf1")
                    nc.sync.dma_start(bf1_l, bff1[:, ds(i, 1), :].rearrange(
                        "p o f -> p (o f)"))
                else:
                    bp_bc = bf2_bc = bf1_l = None

                # ---- attention over each sequence ----
                for b in range(BL):
                    # LN1 + transpose -> hT [128, ET, S]
                    hT = actp.tile([128, ET, S], DT, tag="hT", bufs=1)
                    for j in range(NSEQT):
                        x_t = x_res[b * NSEQT + j]
                        h_t = stat.tile([128, E], DT, tag="hh", bufs=2, name="h1")
                        layernorm_tile(x_t, g1_bc, b1_bc, DT, h_t, ln_trivial)
                        for e in range(ET):
                            transpose_to(hT[:, e, j * 128:(j + 1) * 128],
                                         h_t[:, e * 128:(e + 1) * 128], DT)

                    # QKV
                    qT = actp.tile([128, H, S], DT, tag="qT", bufs=1)
                    kT = actp.tile([128, H, S], DT, tag="kT", bufs=1)
                    for h in range(H):
                        for sc in range(2):
                            w0 = sc * 384
                            for qk in range(2):
                                pq = ps.tile([128, 384], F32, tag="ps")
                                for e in range(ET):
                                    nc.tensor.matmul(
                                        pq, wqk_sb[:, qk, h, e, :],
                                        hT[:, e, w0:w0 + 384],
                                        start=(e == 0), stop=(e == ET - 1))
                                dst = (qT if qk == 0 else kT)[:, h, w0:w0 + 384]
                                nc.scalar.copy(dst, pq)
                    v_sb = actp.tile([128, NSEQT, E], DT, tag="v", bufs=1)
                    for j in range(NSEQT):
                        pv = ps.tile([128, E], F32, tag="ps")
                        for e in range(ET):
                            nc.tensor.matmul(pv, hT[:, e, j * 128:(j + 1) * 128],
                                             wv_sb[:, e, :],
                                             start=(e == 0), stop=(e == ET - 1))
                        nc.scalar.copy(v_sb[:, j, :], pv)

                    # attention per head
                    oT = actp.tile([128, H, S], DT, tag="oT", bufs=1)
                    for h in range(H):
                        att = [actp.tile([128, S - 128 * t_], DT, tag=f"att{t_}", bufs=1,
                                         name=f"att{t_}")
                               for t_ in range(NSEQT)]
                        recip_row = stat.tile([1, S], F32, tag="recip_row")
                        for sc in range(2):
                            s0 = sc * 384
                            ntt = 3 * sc + 3
                            # scores + exp (+ causal mask on diagonal blocks)
                            for tt_ in range(ntt):
                                s_lo = max(s0, 128 * tt_)
                                wdt = s0 + 384 - s_lo
                                pa = ps.tile([128, 384], F32, tag="ps")
                                nc.tensor.matmul(
                                    pa[:, :wdt], kT[:, h, tt_ * 128:(tt_ + 1) * 128],
                                    qT[:, h, s_lo:s_lo + wdt],
                                    start=True, stop=True)
                                dst = att[tt_][:, s_lo - 128 * tt_:
                                               s_lo - 128 * tt_ + wdt]
                                nc.scalar.activation(dst, pa[:, :wdt], AF.Exp,
                                                     scale=SCALE)
                                if s_lo == 128 * tt_:  # diagonal block
                                    nc.gpsimd.affine_select(
                                        dst, dst, pattern=[[1, wdt]],
                                        compare_op=ALU.is_ge, fill=0.0,
                                        base=0, channel_multiplier=-1)
                            # denominator via ones-matmul
                            pd = ps.tile([1, 384], F32, tag="dn", bufs=1)
                            for tt_ in range(ntt):
                                s_lo = max(s0, 128 * tt_)
                                wdt = s0 + 384 - s_lo
                                nc.tensor.matmul(
                                    pd[:, s_lo - s0:s_lo - s0 + wdt], ones_k,
                                    att[tt_][:, s_lo - 128 * tt_:
                                             s_lo - 128 * tt_ + wdt],
                                    start=(tt_ == 0), stop=(tt_ == ntt - 1),
                                    skip_group_check=True)
                            nc.vector.reciprocal(recip_row[:, s0:s0 + 384], pd)
                        recip_bc = actp.tile([128, S], F32, tag="recip_bc", bufs=1)
                        nc.gpsimd.partition_broadcast(recip_bc, recip_row)
                        # o = v.T @ att, normalized on eviction
                        for sc in range(2):
                            s0 = sc * 384
                            ntt = 3 * sc + 3
                            po = ps.tile([128, 384], F32, tag="ps")
                            for tt_ in range(ntt):
                                s_lo = max(s0, 128 * tt_)
                                wdt = s0 + 384 - s_lo
                                nc.tensor.matmul(
                                    po[:, s_lo - s0:s_lo - s0 + wdt],
                                    v_sb[:, tt_, h * HD:(h + 1) * HD],
                                    att[tt_][:, s_lo - 128 * tt_:
                                             s_lo - 128 * tt_ + wdt],
                                    start=(tt_ == 0), stop=(tt_ == ntt - 1),
                                    skip_group_check=True)
                            nc.vector.tensor_tensor(oT[:, h, s0:s0 + 384], po,
                                                    recip_bc[:, s0:s0 + 384],
                                                    op=ALU.mult)

                    # proj + residual -> xm_dram
                    for j in range(NSEQT):
                        pp = psmm.tile([128, E], F32, tag="mm")
                        for h in range(H):
                            nc.tensor.matmul(pp, oT[:, h, j * 128:(j + 1) * 128],
                                             wp_sb[:, h, :],
                                             start=(h == 0), stop=(h == H - 1))
                        xk = x_res[b * NSEQT + j]
                        if bias_trivial:
                            nc.vector.tensor_tensor(xk, pp, xk, op=ALU.add)
                        else:
                            tmpb = xio.tile([128, E], F32, tag="xio")
                            nc.vector.tensor_tensor(tmpb, pp, bp_bc, op=ALU.add)
                            nc.vector.tensor_tensor(xk, tmpb, xk, op=ALU.add)

                # ---- FFN over token chunks ----
                for ch in range(NFCH):
                    h2T = actp.tile([128, ET, FCH], DT, tag="h2T")
                    for j in range(FCH // 128):
                        xm_t = x_res[ch * (FCH // 128) + j]
                        h2_t = stat.tile([128, E], DT, tag="hh", bufs=2, name="h2")
                        layernorm_tile(xm_t, g2_bc, b2_bc, DT, h2_t, ln_trivial)
                        for e in range(ET):
                            transpose_to(h2T[:, e, j * 128:(j + 1) * 128],
                                         h2_t[:, e * 128:(e + 1) * 128], DT)
                    uT = actp.tile([128, FT, FCH], DT, tag="uT", bufs=1)
                    for f in range(FT):
                        pu = ps.tile([128, FCH], F32, tag="ps")
                        for e in range(ET):
                            nc.tensor.matmul(pu, wf1_sb[:, e, f, :], h2T[:, e, :],
                                             start=(e == 0), stop=(e == ET - 1))
                        if bias_trivial:
                            nc.scalar.activation(uT[:, f, :], pu, AF.Relu)
                        else:
                            nc.scalar.activation(uT[:, f, :], pu, AF.Relu,
                                                 bias=bf1_l[:, f:f + 1])
                    for j in range(FCH // 128):
                        py = psmm.tile([128, E], F32, tag="mm")
                        for f in range(FT):
                            nc.tensor.matmul(py, uT[:, f, j * 128:(j + 1) * 128],
                                             wf2_sb[:, f, :],
                                             start=(f == 0), stop=(f == FT - 1))
                        xk = x_res[ch * (FCH // 128) + j]
                        if bias_trivial:
                            nc.vector.tensor_tensor(xk, py, xk, op=ALU.add)
                        else:
                            tmpb = xio.tile([128, E], F32, tag="xio")
                            nc.vector.tensor_tensor(tmpb, py, bf2_bc, op=ALU.add)
                            nc.vector.tensor_tensor(xk, tmpb, xk, op=ALU.add)

            if unroll:
                for i in range(NB):
                    layer_body(i)
            else:
                with tc.For_i(0, NB, 1) as i:
                    layer_body(i)

            # =========================================================
            # final layernorm + heads
            # =========================================================
            for k in range(BL * NSEQT):
                b_, j_ = divmod(k, NSEQT)
                nc.sync.dma_start(x_dram[b_, j_ * 128:(j_ + 1) * 128, :], x_res[k])
            if not ln_trivial:
                gf_bc = load_bcast(lnf_gb[0:1, :], E, "gf", const)
                bf_bc = load_bcast(lnf_gb[1:2, :], E, "bf", const)
            else:
                gf_bc = bf_bc = None
            linS_sb = const.tile([128, ET, OBS], DT)
            nc.sync.dma_start(linS_sb.rearrange("p e n -> p (e n)"), linSw[:])
            linA_sb = const.tile([128, ET, ACTN], DT)
            nc.sync.dma_start(linA_sb.rearrange("p e n -> p (e n)"), linAw[:])
            linRw_bc = load_bcast(linRw[0:1, :], E, "linrw", const)
            hb_bc = load_bcast(hbias[0:1, :], OBS + ACTN + 1, "hbias", const)

            for b in range(BL):
                xv = x_dram[b].rearrange("(t three) e -> three t e", three=3)
                for stream, head in ((2, "RS"), (1, "A")):
                    for j in range(T // 128):
                        x_t = xio.tile([128, E], F32, tag="xio")
                        nc.sync.dma_start(x_t, xv[stream, j * 128:(j + 1) * 128, :])
                        xf = stat.tile([128, E], F32, tag="xf", bufs=2)
                        layernorm_tile(x_t, gf_bc, bf_bc, F32, xf, ln_trivial)
                        if not ln_trivial:
                            pass  # layernorm_tile already applied g/b
                        if head == "RS":
                            # R head on DVE: sum(xf * w) along free dim
                            tmp = stat.tile([128, E], F32, tag="rtmp", bufs=2)
                            yR = stat.tile([128, 1], F32, tag="yR")
                            nc.vector.scalar_tensor_tensor(
                                tmp, xf, 1.0, linRw_bc,
                                op0=ALU.mult, op1=ALU.mult, accum_out=yR)
                            yRb = stat.tile([128, 1], F32, tag="yRb")
                            nc.vector.tensor_tensor(
                                yRb, yR, hb_bc[:, OBS + ACTN:OBS + ACTN + 1],
                                op=ALU.add)
                            nc.sync.dma_start(oR[b, j * 128:(j + 1) * 128, :], yRb)
                        # transpose xf (cast to DT) for the matmul heads
                        xf_c = stat.tile([128, E], DT, tag="xfc", bufs=2)
                        nc.vector.tensor_copy(xf_c, xf)
                        xfT = stat.tile([128, ET, 128], DT, tag="xfT", bufs=2)
                        for e in range(ET):
                            transpose_to(xfT[:, e, :],
                                         xf_c[:, e * 128:(e + 1) * 128], DT)
                        if head == "RS":
                            pS = psmm.tile([128, OBS], F32, tag="mm")
                            for e in range(ET):
                                nc.tensor.matmul(pS, xfT[:, e, :], linS_sb[:, e, :],
                                                 start=(e == 0), stop=(e == ET - 1))
                            yS = stat.tile([128, OBS], F32, tag="yS")
                            nc.vector.tensor_tensor(yS, pS, hb_bc[:, :OBS],
                                                    op=ALU.add)
                            nc.sync.dma_start(oS[b, j * 128:(j + 1) * 128, :], yS)
                        else:
                            pA = psmm.tile([128, ACTN], F32, tag="mm")
                            for e in range(ET):
                                nc.tensor.matmul(pA, xfT[:, e, :], linA_sb[:, e, :],
                                                 start=(e == 0), stop=(e == ET - 1))
                            yA = stat.tile([128, ACTN], F32, tag="yA")
                            nc.vector.tensor_tensor(
                                yA, pA, hb_bc[:, OBS:OBS + ACTN], op=ALU.add)
                            nc.sync.dma_start(oA[b, j * 128:(j + 1) * 128, :], yA)

    nc.compile()
    return nc


_BUILD_CACHE = {}


def _get_nc(ln_trivial, bias_trivial):
    key = (ln_trivial, bias_trivial)
    if key not in _BUILD_CACHE:
        _BUILD_CACHE[key] = build_nc(ln_trivial, bias_trivial)
    return _BUILD_CACHE[key]


def _prep_weights(inputs):
    f32 = lambda x: np.ascontiguousarray(np.asarray(x, dtype=np.float32))
    bf = lambda x: np.ascontiguousarray(
        np.asarray(x, dtype=np.float32).astype(ml_dtypes.bfloat16))
    wq, wk, wv_, pw = (f32(inputs[k]) for k in ("wq", "wk", "wv", "proj_w"))
    ff1, ff2 = f32(inputs["ff1_w"]), f32(inputs["ff2_w"])

    # wqk[i, a, h, e, p, d] = w[i, h, d, e*128+p]
    wqk_np = np.stack([wq, wk], axis=1)          # [NB, 2, H, HD, E]
    wqk_np = wqk_np.transpose(0, 1, 2, 4, 3)      # [NB, 2, H, E, HD]
    wqk_np = wqk_np.reshape(NB, 2, H, ET, 128, HD)
    # wv[i, e, p, n] with n = h*HD+d : wv_[i,h,d,e128+p]
    wv_np = wv_.reshape(NB, E, E).transpose(0, 2, 1).reshape(NB, ET, 128, E)
    # wproj[i, h, p, n] = proj_w[i, n, h*128+p]  (rhs [e, o])
    wp_np = pw.transpose(0, 2, 1).reshape(NB, ET, 128, E)
    # wff1[i, e, f, p, d] = ff1[i, f*128+d, e*128+p]
    wff1_np = ff1.transpose(0, 2, 1).reshape(NB, ET, 128, FF) \
        .reshape(NB, ET, 128, FT, 128).transpose(0, 1, 3, 2, 4)
    # wff2[i, f, p, n] = ff2[i, n, f*128+p]
    wff2_np = ff2.transpose(0, 2, 1).reshape(NB, FT, 128, E)
    # bff1[p, i, f] = ff1_b[i, f*128+p]
    bff1_np = f32(inputs["ff1_b"]).reshape(NB, FT, 128).transpose(2, 0, 1)

    linS_np = f32(inputs["linS_w"]).T.reshape(ET, 128, OBS)   # [e,p,obs]
    linA_np = f32(inputs["linA_w"]).T.reshape(ET, 128, ACTN)

    # flatten to [NB, 128, X] partition-major so DMAs are contiguous
    wqk_np = wqk_np.transpose(0, 4, 1, 2, 3, 5).reshape(NB, 128, -1)
    wv_np = wv_np.transpose(0, 2, 1, 3).reshape(NB, 128, -1)
    wp_np = wp_np.transpose(0, 2, 1, 3).reshape(NB, 128, -1)
    wff1_np = wff1_np.transpose(0, 3, 1, 2, 4).reshape(NB, 128, -1)
    wff2_np = wff2_np.transpose(0, 2, 1, 3).reshape(NB, 128, -1)
    linS_np = linS_np.transpose(1, 0, 2).reshape(128, -1)
    linA_np = linA_np.transpose(1, 0, 2).reshape(128, -1)
    return {
        "se_wT": f32(inputs["se_w"]).T.copy(),                # [OBS, E]
        "wqk": bf(wqk_np), "wv": bf(wv_np), "wproj": bf(wp_np),
        "wff1": bf(wff1_np), "wff2": bf(wff2_np),
        "lng": np.stack([f32(inputs["ln1_g"]), f32(inputs["ln2_g"])], 1),
        "lnb": np.stack([f32(inputs["ln1_b"]), f32(inputs["ln2_b"])], 1),
        "bproj": f32(inputs["proj_b"]), "bff1": bff1_np,
        "bff2": f32(inputs["ff2_b"]),
        "lnf_gb": np.stack([f32(inputs["lnf_g"]), f32(inputs["lnf_b"])], 0),
        "linSw": bf(linS_np), "linAw": bf(linA_np),
        "linRw": f32(inputs["linR_w"]).reshape(1, E),
        "hbias": np.concatenate([f32(inputs["linS_b"]), f32(inputs["linA_b"]),
                                 f32(inputs["linR_b"])]).reshape(1, -1),
    }


def _detect_flags(inputs):
    ln_trivial = all(
        np.all(np.asarray(inputs[k], np.float32) == 1.0) for k in
        ("ln1_g", "ln2_g", "lnf_g")) and all(
        np.all(np.asarray(inputs[k], np.float32) == 0.0) for k in
        ("ln1_b", "ln2_b", "lnf_b"))
    bias_trivial = all(
        np.all(np.asarray(inputs[k], np.float32) == 0.0) for k in
        ("proj_b", "ff1_b", "ff2_b"))
    return ln_trivial, bias_trivial


def _make_in_maps(inputs):
    R = np.asarray(inputs["R"], dtype=np.float32)
    s = np.asarray(inputs["s"], dtype=np.float32)
    a = np.asarray(inputs["a"]).astype(np.int64)
    t = np.asarray(inputs["t"]).astype(np.int64)
    pos_emb = np.asarray(inputs["pos_emb"], dtype=np.float32)
    act_emb = np.asarray(inputs["act_emb"], dtype=np.float32)

    w = _prep_weights(inputs)

    # host-side embedding prep (pure gathers / elementwise; the se matmul
    # runs on device)
    pos = pos_emb[t]                                   # [B, T, E]
    re_w = np.asarray(inputs["re_w"], dtype=np.float32)
    re_b = np.asarray(inputs["re_b"], dtype=np.float32)
    se_b = np.asarray(inputs["se_b"], dtype=np.float32)
    re = R * re_w[:, 0][None, None, :] + re_b + pos    # [B, T, E]
    ae = act_emb[a] + pos
    se_part = se_b[None, None, :] + pos                # se matmul added on device
    x0 = np.stack([re, se_part, ae], axis=2).reshape(B, S, E).astype(np.float32)

    head_bias = np.concatenate([
        np.asarray(inputs["linS_b"], np.float32).ravel(),
        np.asarray(inputs["linA_b"], np.float32).ravel(),
        np.asarray(inputs["linR_b"], np.float32).ravel()]).reshape(1, -1)
    w["hbias"] = head_bias

    in_maps = []
    for c in range(N_CORES):
        bsl = slice(c * BL, (c + 1) * BL)
        m = dict(w)
        m["x0"] = np.ascontiguousarray(x0[bsl])
        m["sT"] = np.ascontiguousarray(s[bsl].transpose(0, 2, 1))
        in_maps.append(m)
    return in_maps


def kernel(**inputs):
    in_maps = _make_in_maps(inputs)
    nc = _get_nc(*_detect_flags(inputs))
    res = run_bass_kernel_spmd(nc, in_maps, core_ids=list(range(N_CORES)))
    outR = np.concatenate([res.results[c]["oR"] for c in range(N_CORES)], 0)
    outS = np.concatenate([res.results[c]["oS"] for c in range(N_CORES)], 0)
    outA = np.concatenate([res.results[c]["oA"] for c in range(N_CORES)], 0)
    return outR, outS, outA
